# revision 8
# baseline (speedup 1.0000x reference)
"""Bahdanau attention Trainium2 kernel.

reference:
    w1_eo = einsum("bse,eu->bsu", EO, W1) + b1
    w2_h  = (H @ W2 + b2)[:, None, :]
    score = tanh(w1_eo + w2_h) @ V + bv          # [B, S, 1]
    attn  = softmax(score, axis=1)
    ctx   = sum(attn * EO, axis=1)               # [B, E]
    returns (ctx, attn)

Sharding: data-parallel over B across 8 cores (8 batch rows per core).
Per-core plan (mapping "z^T"):
  - EO tiles loaded via SWDGE cast-DMA fp32->bf16 into a persistent natural
    SBUF buffer [s-part, e-free] (later the rhs of the context matmul).
  - DMA-xbar transpose (bf16) gives E-major tiles [e-part, s-free], the rhs
    of the score matmul z^T[u, s] = sum_e W1[e, u] EO^T[e, s].
  - tanh on ScalarE with per-partition bias (W2 H + b1 + b2)[u], bf16 out.
  - score[s] = V^T tanh(z) on PE (m=1); exp on ScalarE with fused free-dim
    accumulation for the softmax denominator. Scores are bounded
    (|score| <= ||V||_1 + |bv| ~ 20) so unnormalized exp is safe, and the
    scalar bv cancels in the softmax, so it is skipped entirely.
  - ctx_raw = sum_s exp[s] * EO[s, :] on PE with exp (bf16, transposed to
    s-part via DMA xbar) stationary; one divide by sum(exp) at the end.

Engine ops only address SBUF partition starts {0,32,64,96} (<=32 rows for
nonzero starts), so per-batch scalars/rows for batch b live at partition
32*(b%4), column group b//4; the m=1 matmuls target those partitions with
tile_position=(0, 32*(b%4)).
"""

import os
from functools import lru_cache

import numpy as np

import concourse.bass as bass
import concourse.tile as tile
import concourse.bacc as bacc
from concourse import mybir
from concourse.bass import ts
from concourse.bass_utils import run_bass_kernel_spmd

F32 = mybir.dt.float32
BF16 = mybir.dt.bfloat16
AF = mybir.ActivationFunctionType

N_CORES = 8
B, S, E, U = 64, 2048, 512, 512
NB = B // N_CORES  # batch rows per core


def build_nc(nb=NB, s=S, repeat=1, use_cast_dma=True):
    """Build + compile the per-core program. nb = local batch rows, s = seq len."""
    nt = s // 512          # 512-wide s-tiles per batch row
    ng = (nb + 3) // 4     # column groups of 4 batch rows

    nc = bacc.Bacc("TRN2", target_bir_lowering=False, debug=False)

    eo_d = nc.dram_tensor("EO", [nb, s, E], F32, kind="ExternalInput").ap()
    h_d = nc.dram_tensor("H", [nb, E], F32, kind="ExternalInput").ap()
    w1_d = nc.dram_tensor("W1", [E, U], F32, kind="ExternalInput").ap()
    b1_d = nc.dram_tensor("b1", [U], F32, kind="ExternalInput").ap()
    w2_d = nc.dram_tensor("W2", [E, U], F32, kind="ExternalInput").ap()
    b2_d = nc.dram_tensor("b2", [U], F32, kind="ExternalInput").ap()
    v_d = nc.dram_tensor("V", [U, 1], F32, kind="ExternalInput").ap()
    ctx_d = nc.dram_tensor("ctx", [nb, E], F32, kind="ExternalOutput").ap()
    attn_d = nc.dram_tensor("attn", [nb, s], F32, kind="ExternalOutput").ap()

    def row(b):
        return 32 * (b % 4)

    def grp(b):
        return b // 4

    with tile.TileContext(nc) as tc:
        with tc.tile_pool(name="persist", bufs=1) as pp:
            # ---- persistent SBUF ----
            eo_bf = pp.tile([128, nb * nt * 4 * 512], BF16)  # natural bf16 EO
            w1_bf = pp.tile([128, 4 * 512], BF16)         # [e-part, (e-chunk, u)]
            w2_bf = pp.tile([128, 4 * 512], BF16)
            v_bf = pp.tile([128, 4], BF16)                # u-chunk columns
            b12 = pp.tile([128, 4], F32)                  # b1+b2, u-chunk cols
            bias_all = pp.tile([128, 4 * nb], F32)        # (W2H^T + b12), [u, j*nb+b]
            hpad = pp.tile([16, E], BF16)                 # H bf16, padded rows
            hT = pp.tile([128, 4 * 16], BF16)             # H^T blocks
            exp_f32 = pp.tile([128, ng * s], F32)         # exp at [row(b), grp(b)*s + s_idx]
            pad2 = pp.tile([128, s], BF16)                # bf16 exp staging for transpose
            acc = pp.tile([128, ng * nt], F32)            # per (b, t) exp partial sums
            ctx_raw = pp.tile([128, ng * E], F32)
            sums = pp.tile([128, ng], F32)
            recip = pp.tile([128, ng], F32)

            # ---- setup: weights ----
            with tc.tile_pool(name="setup", bufs=1) as stage:
                if use_cast_dma:
                    nc.gpsimd.dma_start(
                        w1_bf[:].rearrange("p (c u) -> p c u", c=4),
                        w1_d.rearrange("(c p) u -> p c u", p=128),
                    )
                    nc.gpsimd.dma_start(
                        w2_bf[:].rearrange("p (c u) -> p c u", c=4),
                        w2_d.rearrange("(c p) u -> p c u", p=128),
                    )
                else:
                    wtmp = stage.tile([128, 4 * 512], F32, tag="w1tmp")
                    nc.sync.dma_start(
                        wtmp[:].rearrange("p (c u) -> p c u", c=4),
                        w1_d.rearrange("(c p) u -> p c u", p=128),
                    )
                    nc.vector.tensor_copy(w1_bf[:], wtmp[:])
                    wtmp2 = stage.tile([128, 4 * 512], F32, tag="w2tmp")
                    nc.sync.dma_start(
                        wtmp2[:].rearrange("p (c u) -> p c u", c=4),
                        w2_d.rearrange("(c p) u -> p c u", p=128),
                    )
                    nc.vector.tensor_copy(w2_bf[:], wtmp2[:])

                vtmp = stage.tile([128, 4], F32, tag="vtmp")
                b1t = stage.tile([128, 4], F32, tag="b1t")
                b2t = stage.tile([128, 4], F32, tag="b2t")
                for c in range(4):
                    nc.sync.dma_start(vtmp[:, c : c + 1], v_d[ts(c, 128), :])
                    nc.sync.dma_start(
                        b1t[:, c : c + 1],
                        b1_d[ts(c, 128)].rearrange("(p q) -> p q", q=1),
                    )
                    nc.sync.dma_start(
                        b2t[:, c : c + 1],
                        b2_d[ts(c, 128)].rearrange("(p q) -> p q", q=1),
                    )
                nc.vector.tensor_copy(v_bf[:], vtmp[:])
                nc.vector.tensor_add(b12[:], b1t[:], b2t[:])

                # ---- setup: H^T and W2 H ----
                htmp = stage.tile([nb, E], F32, tag="htmp")
                nc.sync.dma_start(htmp[:], h_d[:])
                nc.gpsimd.memset(hpad[:], 0.0)
                nc.vector.tensor_copy(hpad[0:nb, :], htmp[:])
                for c in range(4):
                    nc.sync.dma_start(
                        hT[:, ts(c, 16)], hpad[:, ts(c, 128)], transpose=True
                    )

                with tc.tile_pool(name="psetup", bufs=1, space="PSUM") as psetup:
                    for j in range(4):
                        w2h = psetup.tile([128, nb], F32, tag="w2h")
                        for c in range(4):
                            nc.tensor.matmul(
                                w2h[:],
                                w2_bf[:, c * 512 + j * 128 : c * 512 + (j + 1) * 128],
                                hT[:, c * 16 : c * 16 + nb],
                                start=(c == 0),
                                stop=(c == 3),
                            )
                        nc.vector.tensor_scalar_add(
                            bias_all[:, ts(j, nb)], w2h[:], b12[:, j : j + 1]
                        )

            nc.gpsimd.memset(pad2[:], 0.0)
            nc.gpsimd.memset(acc[:], 1.0)
            nc.gpsimd.memset(exp_f32[:], 0.0)
            nc.gpsimd.memset(ctx_raw[:], 0.0)

            # ---- main loop ----
            with (
                tc.tile_pool(name="eoT", bufs=2) as eoT_pool,
                tc.tile_pool(name="tanh", bufs=2) as tanh_pool,
                tc.tile_pool(name="wt", bufs=2) as wt_pool,
                tc.tile_pool(name="stage2", bufs=3) as stage2,
                tc.tile_pool(name="pz", bufs=1, space="PSUM") as pz_pool,
                tc.tile_pool(name="psc", bufs=2, space="PSUM") as psc_pool,
                tc.tile_pool(name="pctx", bufs=2, space="PSUM") as pctx_pool,
            ):
                for rep in range(repeat):
                    for b in range(nb):
                        r, g = row(b), grp(b)
                        if b % 4 == 0:
                            ctx_ps = pctx_pool.tile([128, E], F32, tag="ctx", name="ctx_ps")
                        for t in range(nt):
                            st = b * nt + t

                            def ecol(j):
                                return (st * 4 + j) * 512

                            # 1) load + cast natural EO tile
                            for j in range(4):
                                src = eo_d[
                                    b, t * 512 + j * 128 : t * 512 + (j + 1) * 128, :
                                ]
                                if use_cast_dma:
                                    nc.gpsimd.dma_start(
                                        eo_bf[:, ecol(j) : ecol(j) + 512], src
                                    )
                                else:
                                    st_t = stage2.tile(
                                        [128, 512], F32, tag="eostage", name="eostage"
                                    )
                                    nc.sync.dma_start(st_t[:], src)
                                    nc.vector.tensor_copy(
                                        eo_bf[:, ecol(j) : ecol(j) + 512], st_t[:]
                                    )

                            # 2) transpose to E-major
                            eoT = [
                                eoT_pool.tile(
                                    [128, 512], BF16, tag=f"eoT{c}", name=f"eoT{c}"
                                )
                                for c in range(4)
                            ]
                            for c in range(4):
                                for j in range(4):
                                    nc.sync.dma_start(
                                        eoT[c][:, ts(j, 128)],
                                        eo_bf[
                                            :,
                                            ecol(j) + c * 128 : ecol(j) + (c + 1) * 128,
                                        ],
                                        transpose=True,
                                    )

                            # 3) z^T = W1^T EO^T  (accumulate over e-chunks)
                            zt = [
                                pz_pool.tile([128, 512], F32, tag=f"z{j}", name=f"z{j}")
                                for j in range(4)
                            ]
                            for j in range(4):
                                for c in range(4):
                                    nc.tensor.matmul(
                                        zt[j][:],
                                        w1_bf[
                                            :, c * 512 + j * 128 : c * 512 + (j + 1) * 128
                                        ],
                                        eoT[c][:],
                                        start=(c == 0),
                                        stop=(c == 3),
                                    )

                            # 4) tanh(z + bias) -> bf16
                            th = [
                                tanh_pool.tile(
                                    [128, 512], BF16, tag=f"th{j}", name=f"th{j}"
                                )
                                for j in range(4)
                            ]
                            for j in range(4):
                                nc.scalar.activation(
                                    th[j][:],
                                    zt[j][:],
                                    AF.Tanh,
                                    bias=bias_all[:, j * nb + b : j * nb + b + 1],
                                )

                            # 5) score = V^T tanh  (at partition row(b))
                            sc = psc_pool.tile([128, 512], F32, tag="sc", name="sc")
                            for j in range(4):
                                nc.tensor.matmul(
                                    sc[r : r + 1, :],
                                    v_bf[:, j : j + 1],
                                    th[j][:],
                                    start=(j == 0),
                                    stop=(j == 3),
                                    tile_position=(0, r),
                                )

                            # 6) exp (unnormalized) + partial sum
                            nc.scalar.activation(
                                exp_f32[r : r + 1, g * s + t * 512 : g * s + (t + 1) * 512],
                                sc[r : r + 1, :],
                                AF.Exp,
                                accum_out=acc[r : r + 1, g * nt + t : g * nt + t + 1],
                            )

                            # 7) bf16 copy for the context matmul stationary
                            nc.vector.tensor_copy(
                                pad2[r : r + 1, ts(t, 512)],
                                exp_f32[r : r + 1, g * s + t * 512 : g * s + (t + 1) * 512],
                            )

                            # 8) transpose exp to s-part layout
                            wt = [
                                wt_pool.tile([128, 16], BF16, tag=f"wt{k}", name=f"wt{k}")
                                for k in range(4)
                            ]
                            for k in range(4):
                                nc.sync.dma_start(
                                    wt[k][:],
                                    pad2[r : r + 16, t * 512 + k * 128 : t * 512 + (k + 1) * 128],
                                    transpose=True,
                                )

                            # 9) ctx_raw += exp^T @ EO   (at partition row(b))
                            for k in range(4):
                                nc.tensor.matmul(
                                    ctx_ps[r : r + 1, :],
                                    wt[k][:, 0:1],
                                    eo_bf[:, ecol(k) : ecol(k) + 512],
                                    start=(t == 0 and k == 0),
                                    stop=(t == nt - 1 and k == 3),
                                    tile_position=(0, r),
                                )

                        nc.vector.tensor_copy(
                            ctx_raw[r : r + 1, ts(g, E)], ctx_ps[r : r + 1, :]
                        )

                # ---- epilogue ----
                nc.vector.tensor_reduce(
                    sums[:],
                    acc[:].rearrange("p (g t) -> p g t", g=ng),
                    mybir.AxisListType.X,
                    mybir.AluOpType.add,
                )
                nc.vector.reciprocal(recip[:], sums[:])
                for g in range(ng):
                    nc.vector.tensor_scalar_mul(
                        exp_f32[:, ts(g, s)], exp_f32[:, ts(g, s)], recip[:, g : g + 1]
                    )
                    nc.vector.tensor_scalar_mul(
                        ctx_raw[:, ts(g, E)], ctx_raw[:, ts(g, E)], recip[:, g : g + 1]
                    )
                for b in range(nb):
                    r, g = row(b), grp(b)
                    nc.sync.dma_start(
                        attn_d[b : b + 1, :], exp_f32[r : r + 1, ts(g, s)]
                    )
                    nc.sync.dma_start(
                        ctx_d[b : b + 1, :], ctx_raw[r : r + 1, ts(g, E)]
                    )

    nc.compile()
    return nc


@lru_cache(maxsize=2)
def _built(repeat=1):
    return build_nc(repeat=repeat)


def kernel(H, EO, W1, b1, W2, b2, V, bv):
    H = np.ascontiguousarray(H, dtype=np.float32)
    EO = np.ascontiguousarray(EO, dtype=np.float32)
    nc = _built(int(os.environ.get("KERNEL_REPEAT", "1")))
    in_maps = []
    for i in range(N_CORES):
        sl = slice(i * NB, (i + 1) * NB)
        in_maps.append(
            {
                "H": H[sl],
                "EO": EO[sl],
                "W1": np.ascontiguousarray(W1, dtype=np.float32),
                "b1": np.ascontiguousarray(b1, dtype=np.float32),
                "W2": np.ascontiguousarray(W2, dtype=np.float32),
                "b2": np.ascontiguousarray(b2, dtype=np.float32),
                "V": np.ascontiguousarray(V, dtype=np.float32),
            }
        )
    res = run_bass_kernel_spmd(nc, in_maps, core_ids=list(range(N_CORES)))
    ctx = np.concatenate([res.results[i]["ctx"] for i in range(N_CORES)], axis=0)
    attn = np.concatenate([res.results[i]["attn"] for i in range(N_CORES)], axis=0)
    return ctx.astype(np.float32), attn.reshape(B, S, 1).astype(np.float32)
